# revision 4
# baseline (speedup 1.0000x reference)
"""CaNet GAT-style layer on 8 Trainium2 NeuronCores — v2 (fp8 + dma_gather).

Algorithm (matches the jax reference):
  h[k]   = x @ W[k]                      per-head projection
  s_src  = x @ (W[k] a_src[k]),  s_dst = x @ (W[k] a_dst[k])
  w_e    = exp(leakyrelu(s_src[src] + s_dst[dst]) - 2)   (softmax shift
           invariance; the -2 keeps w*h inside fp8-e4m3 range)
  hi[k,i] = sum_{e: src[e]=i} w_e * h[k, dst[e]]
  den[k,i]= sum_{e: src[e]=i} w_e
  out[i] = sum_k e[i,k] * hi[k,i]/den[k,i] + x[i]

v2 vs v1: node table in fp8 (768-B rows: 512 B h + 16 B s_dst f32 + pad),
edge rows fetched with batched dma_gather (one call per (block, window),
int16 indices; two 32768-row windows cover the 50048-row table), one-hot
AND transposed one-hot shipped from host in fp8, per-block s_src
recomputed on the PE from xmT @ (W a_src).
"""

import heapq
import math
import os
import sys
from contextlib import ExitStack

import numpy as np

for _p in ("/opt/trn_rl_repo", "/root/.axon_site/_ro/trn_rl_repo"):
    if os.path.isdir(_p) and _p not in sys.path:
        sys.path.insert(0, _p)

import ml_dtypes

import concourse.bass as bass
import concourse.mybir as mybir
import concourse.tile as tile
from concourse import bacc
from concourse.bass_utils import run_bass_kernel_spmd
from concourse.masks import make_identity

N = 50000
F = 128
K = 4
NCORES = 8
BLK = 128
NBLK = 49                    # blocks per core (49*128 = 6272 >= 6250)
NPAD = NBLK * BLK            # 6272
NCHUNK = math.ceil(N / 128)  # 391
TROWS = NCHUNK * 128         # 50048
ROWB = 768                   # fp8 bytes/row: [0:512) h, [512:528) s_dst f32[4]
WIN = 32768                  # dma_gather int16 index window
W1B = TROWS - WIN            # 17280 = base of window 1
ALPHA = 0.01
EPS = 1e-8
SHIFT = 2.0

f32 = mybir.dt.float32
bf16 = mybir.dt.bfloat16
i16 = mybir.dt.int16
f8 = mybir.dt.float8e4
np_f8 = ml_dtypes.float8_e4m3

_CACHE = {}


def _prep(x, e, weights, a, adj):
    """Host-side sharding/indexing prep. Only touches adj for structure;
    all floating point math happens on device (x is only cast to bf16)."""
    src = np.concatenate([np.asarray(adj[0]), np.arange(N, dtype=np.int64)])
    dst = np.concatenate([np.asarray(adj[1]), np.arange(N, dtype=np.int64)])

    # degree-balanced assignment of nodes to (core, block) bins
    NBINS = NCORES * NBLK
    deg = np.bincount(src, minlength=N)
    order0 = np.argsort(-deg, kind="stable")
    heap = [(0, b) for b in range(NBINS)]
    heapq.heapify(heap)
    bcounts = np.zeros(NBINS, dtype=np.int64)
    assign = np.empty(N, dtype=np.int64)
    pos = np.empty(N, dtype=np.int64)
    for nd in order0:
        while True:
            s_, b_ = heapq.heappop(heap)
            if bcounts[b_] < BLK:
                break
        assign[nd] = b_
        pos[nd] = bcounts[b_]
        bcounts[b_] += 1
        if bcounts[b_] < BLK:
            heapq.heappush(heap, (s_ + int(deg[nd]), b_))

    core_of_n = assign // NBLK
    loc = (assign % NBLK) * BLK + pos

    ecore = core_of_n[src]
    eblk = assign[src] % NBLK
    esrcrel = pos[src]
    # window class: 0 forced, 1 forced, 2 = flexible (overlap region)
    wclass = np.where(dst < W1B, 0, np.where(dst >= WIN, 1, 2)).astype(np.int8)

    # group edges by (core, block); within: forced-w0, flex, forced-w1
    wkey = np.where(wclass == 1, 2, np.where(wclass == 2, 1, 0)).astype(np.int8)
    order = np.lexsort((dst, wkey, eblk, ecore))
    g_core = ecore[order]
    g_blk = eblk[order]
    g_src = esrcrel[order]
    g_dst = dst[order]
    g_wc = wclass[order]

    key = g_core * NBLK + g_blk
    counts = np.bincount(key, minlength=NBINS)
    starts = np.zeros(NBINS + 1, dtype=np.int64)
    np.cumsum(counts, out=starts[1:])

    # per (core, block): n0 range [lo, lo+flex]
    lo_cb = np.zeros(NBINS, dtype=np.int64)
    fl_cb = np.zeros(NBINS, dtype=np.int64)
    for cb in range(NBINS):
        s0, s1 = starts[cb], starts[cb + 1]
        wc = g_wc[s0:s1]
        lo_cb[cb] = int((wc == 0).sum())
        fl_cb[cb] = int((wc == 2).sum())

    # uniform (t0, t1) per block slot across cores
    t0 = np.zeros(NBLK, dtype=np.int64)
    t1 = np.zeros(NBLK, dtype=np.int64)
    for b in range(NBLK):
        ns = counts[b::NBLK]
        los = lo_cb[b::NBLK]
        his = los + fl_cb[b::NBLK]
        T = int(math.ceil(ns.max() / 128))
        found = False
        while not found:
            # prefer balanced splits (bounds the per-call SBUF tile)
            for d in range(T + 1):
                for a0 in ({(T + 1) // 2 - d, (T + 1) // 2 + d} if d else
                           {(T + 1) // 2}):
                    if not (0 <= a0 <= T):
                        continue
                    a1 = T - a0
                    lof = np.maximum(los, ns - 128 * a1)
                    hif = np.minimum(his, 128 * a0)
                    if (lof <= hif).all():
                        t0[b], t1[b] = a0, a1
                        found = True
                        break
                if found:
                    break
            if not found:
                T += 1

    S_TOT = int((t0 + t1).sum())
    calls = []
    s0_off = 0
    for b in range(NBLK):
        present = [w for w, t in ((0, t0[b]), (1, t1[b])) if t > 0]
        for w in present:
            t = int(t0[b] if w == 0 else t1[b])
            calls.append(dict(b=b, w=w, t=t, s0=s0_off,
                              first=(w == present[0]), last=(w == present[-1])))
            s0_off += t
    max_t = int(max(t0.max(), t1.max()))

    Ah = np.zeros((NCORES, 128, S_TOT * 128), dtype=np_f8)
    AhT = np.zeros((NCORES, 128, S_TOT * 128), dtype=np_f8)
    DI = np.zeros((NCORES, 128, S_TOT * 8), dtype=np.int16)

    sub_of = {(cl["b"], cl["w"]): cl["s0"] for cl in calls}
    for c in range(NCORES):
        for b in range(NBLK):
            cb = c * NBLK + b
            s0, s1 = starts[cb], starts[cb + 1]
            n = s1 - s0
            lof = max(lo_cb[cb], n - 128 * t1[b])
            hif = min(lo_cb[cb] + fl_cb[cb], 128 * t0[b])
            assert lof <= hif
            n0 = int(hif)  # maximize w0 fill
            for w, lo_e, hi_e in ((0, s0, s0 + n0), (1, s0 + n0, s1)):
                ne = hi_e - lo_e
                if ne == 0:
                    continue
                sb_ = sub_of[(b, w)]
                j = np.arange(ne)
                m = j // 128
                p = j % 128
                sr = g_src[lo_e:hi_e]
                dr = g_dst[lo_e:hi_e] - (0 if w == 0 else W1B)
                Ah[c, p, (sb_ + m) * 128 + sr] = 1.0
                AhT[c, sr, (sb_ + m) * 128 + p] = 1.0
                t = int(t0[b] if w == 0 else t1[b])
                il = np.zeros(t * 128, dtype=np.int16)
                il[m * 128 + p] = dr
                blk16 = il.reshape(t * 8, 16).T  # [16, t*8]
                DI[c, :, sb_ * 8:(sb_ + t) * 8] = np.tile(blk16, (8, 1))

    # combined per-call [asb | atb] regions: cols [s0*256, (s0+t)*256)
    AHB = np.zeros((NCORES, 128, S_TOT * 256), dtype=np_f8)
    for cl in calls:
        s0, t = cl["s0"], cl["t"]
        AHB[:, :, s0 * 256:s0 * 256 + t * 128] = \
            Ah[:, :, s0 * 128:(s0 + t) * 128]
        AHB[:, :, s0 * 256 + t * 128:(s0 + t) * 256] = \
            AhT[:, :, s0 * 128:(s0 + t) * 128]

    xtb = np.zeros((128, TROWS), dtype=ml_dtypes.bfloat16)
    xtb[:, :N] = x.T
    xm = np.zeros((NCORES, NPAD, F), dtype=np.float32)
    em = np.zeros((NCORES, NPAD, K), dtype=np.float32)
    xmt = np.zeros((NCORES, 128, NPAD), dtype=ml_dtypes.bfloat16)
    xm[core_of_n, loc] = x
    em[core_of_n, loc] = e
    for c in range(NCORES):
        xmt[c] = xm[c].T

    meta = dict(calls=calls, S_TOT=S_TOT, max_t=max_t,
                key=(tuple(int(v) for v in t0), tuple(int(v) for v in t1)),
                core_of=core_of_n, loc=loc)
    in_maps = []
    for c in range(NCORES):
        in_maps.append({
            "xtb": xtb,
            "w": np.ascontiguousarray(np.asarray(weights, dtype=np.float32)),
            "a": np.ascontiguousarray(np.asarray(a, dtype=np.float32)),
            "ahb": np.ascontiguousarray(AHB[c]),
            "di": np.ascontiguousarray(DI[c]),
            "xm": np.ascontiguousarray(xm[c]),
            "xmt": np.ascontiguousarray(xmt[c]),
            "em": np.ascontiguousarray(em[c]),
        })
    return meta, in_maps


def _build(meta, repeat=1):
    calls, S_TOT, MAXT = meta["calls"], meta["S_TOT"], meta["max_t"]
    nc = bacc.Bacc(num_swdge_queues=2)

    XTb = nc.declare_dram_parameter("xtb", [128, TROWS], bf16, isOutput=False)
    W = nc.declare_dram_parameter("w", [K, 128, 128], f32, isOutput=False)
    Aa = nc.declare_dram_parameter("a", [K, 256, 1], f32, isOutput=False)
    AHB = nc.declare_dram_parameter("ahb", [128, S_TOT * 256], f8, isOutput=False)
    DI = nc.declare_dram_parameter("di", [128, S_TOT * 8], i16, isOutput=False)
    XM = nc.declare_dram_parameter("xm", [NPAD, F], f32, isOutput=False)
    XMT = nc.declare_dram_parameter("xmt", [128, NPAD], bf16, isOutput=False)
    EM = nc.declare_dram_parameter("em", [NPAD, K], f32, isOutput=False)
    OUT = nc.declare_dram_parameter("out", [NPAD, F], f32, isOutput=True)

    Copy = mybir.ActivationFunctionType.Copy
    Exp = mybir.ActivationFunctionType.Exp
    Lrelu = mybir.ActivationFunctionType.Lrelu

    with ExitStack() as ctx:
        tc = ctx.enter_context(tile.TileContext(nc))
        if repeat > 1:
            # timing-only: repeat the whole body on-device (see bench3)
            ctx.enter_context(tc.For_i(0, repeat, name="rep"))
        const = ctx.enter_context(tc.tile_pool(name="const", bufs=1))
        sb = ctx.enter_context(tc.tile_pool(name="sb", bufs=3))
        sbG = ctx.enter_context(tc.tile_pool(name="sbG", bufs=3))
        fin = ctx.enter_context(tc.tile_pool(name="fin", bufs=2))
        dram = ctx.enter_context(tc.tile_pool(name="dram", bufs=1, space="DRAM"))

        Ttab = dram.tile([TROWS, ROWB], f8, tag="Ttab")

        ident = const.tile([128, 128], f32, tag="ident")
        make_identity(nc, ident[:])
        shiftc = const.tile([128, 1], f32, tag="shiftc")
        nc.gpsimd.memset(shiftc[:], -SHIFT)

        # ---- prologue: W01/W23 bf16, AVs/AVd = W_k @ a_{src,dst}[k], ssb ----
        W01 = const.tile([128, 256], bf16, tag="W01")
        W23AV = const.tile([128, 260], bf16, tag="W23AV")
        AVs = const.tile([128, K], bf16, tag="AVs")
        ssb = []
        with tc.tile_pool(name="psP", bufs=1, space="PSUM") as psP:
            for k in range(K):
                wk = sb.tile([128, 128], f32, tag="wk")
                nc.sync.dma_start(out=wk[:], in_=W[k])
                tgt = W01 if k < 2 else W23AV
                j = (k % 2) * 128
                nc.vector.tensor_copy(out=tgt[:, j:j + 128], in_=wk[:])
                ak = sb.tile([128, 2], f32, tag="ak")
                nc.sync.dma_start(out=ak[:, 0:1], in_=Aa[k, 0:128, :])
                nc.sync.dma_start(out=ak[:, 1:2], in_=Aa[k, 128:256, :])
                pT = psP.tile([128, 128], f32, tag="pT")
                nc.tensor.transpose(pT[:], wk[:], ident[:])
                wkT = sb.tile([128, 128], f32, tag="wkT")
                nc.scalar.copy(out=wkT[:], in_=pT[:])
                pva = psP.tile([128, 2], f32, tag="pva")
                nc.tensor.matmul(pva[:], wkT[:], ak[:], start=True, stop=True)
                nc.vector.tensor_copy(out=AVs[:, k:k + 1], in_=pva[:, 0:1])
                nc.vector.tensor_copy(out=W23AV[:, 256 + k:257 + k],
                                      in_=pva[:, 1:2])
            xmt = const.tile([128, NPAD], bf16, tag="xmt")
            nc.sync.dma_start(out=xmt[:], in_=XMT[:, :])
            diall = const.tile([128, S_TOT * 8], i16, tag="diall")
            nc.sync.dma_start(out=diall[:], in_=DI[:, :])
            xmall = const.tile([128, NBLK * F], f32, tag="xmall")
            nc.sync.dma_start(
                out=xmall[:].rearrange("p (b f) -> p b f", f=F),
                in_=XM[:, :].rearrange("(b p) f -> p b f", p=128))
            emall = const.tile([128, NBLK * K], f32, tag="emall")
            nc.sync.dma_start(
                out=emall[:].rearrange("p (b f) -> p b f", f=K),
                in_=EM[:, :].rearrange("(b p) f -> p b f", p=128))
            outall = const.tile([128, NBLK * F], f32, tag="outall")
            for b in range(NBLK):
                psS = psP.tile([128, K], f32, tag="psS")
                nc.tensor.matmul(psS[:], xmt[:, b * 128:(b + 1) * 128], AVs[:],
                                 start=True, stop=True)
                sb_b = const.tile([128, K], bf16, tag=f"ssb{b}")
                nc.scalar.copy(out=sb_b[:], in_=psS[:])
                ssb.append(sb_b)

        # ---- phase A: build node table Ttab (fp8 rows) ----
        with tc.tile_pool(name="psA", bufs=2, space="PSUM") as psA:
            for c4 in range(0, NCHUNK, 4):
                w4 = min(4, NCHUNK - c4)
                xc = sb.tile([128, 512], bf16, tag="xc")
                nc.scalar.dma_start(out=xc[:, 0:w4 * 128],
                                    in_=XTb[:, c4 * 128:(c4 + w4) * 128])
                tsb = sb.tile([128, 4 * ROWB], f8, tag="tsb")
                for cc in range(w4):
                    lhs = xc[:, cc * 128:(cc + 1) * 128]
                    o = cc * ROWB
                    pA = psA.tile([128, 256], f32, tag="pA")
                    pB = psA.tile([128, 260], f32, tag="pB")
                    nc.tensor.matmul(pA[:], lhs, W01[:], start=True, stop=True)
                    nc.tensor.matmul(pB[:], lhs, W23AV[:], start=True, stop=True)
                    nc.scalar.copy(out=tsb[:, o:o + 256], in_=pA[:])
                    nc.vector.tensor_copy(out=tsb[:, o + 256:o + 512],
                                          in_=pB[:, 0:256])
                    # s_dst f32[4] + pad filled with copies of it (one op)
                    nc.vector.tensor_copy(
                        out=tsb[:, o + 512:o + ROWB].bitcast(f32)
                            .rearrange("p (g f) -> p g f", f=4),
                        in_=pB[:, None, 256:260].to_broadcast([128, 16, 4]))
                nc.sync.dma_start(
                    out=Ttab[c4 * 128:(c4 + w4) * 128, :]
                        .rearrange("(q p) r -> p q r", p=128),
                    in_=tsb[:, 0:w4 * ROWB]
                        .rearrange("p (q r) -> p q r", r=ROWB))

        # ---- phase B: gather + attention + segment reduction ----
        with tc.tile_pool(name="psHI", bufs=2, space="PSUM") as psHI, \
                tc.tile_pool(name="psG", bufs=2, space="PSUM") as psG:
            curA = curB = None
            for ci, cl in enumerate(calls):
                b, w, t, s0 = cl["b"], cl["w"], cl["t"], cl["s0"]
                base = 0 if w == 0 else W1B
                ab = sbG.tile([128, MAXT * 256], f8, tag="ab")
                nc.sync.dma_start(out=ab[:, 0:t * 256],
                                  in_=AHB[:, s0 * 256:(s0 + t) * 256])
                asb = ab[:, 0:t * 128]
                atb = ab[:, t * 128:t * 256]
                graw = sbG.tile([128, MAXT * ROWB], f8, tag="graw")
                gv = graw[:, 0:t * ROWB].rearrange("p (m r) -> p m r", r=ROWB)
                nc.gpsimd.dma_gather(
                    out_ap=gv, in_ap=Ttab[base:base + WIN, :],
                    idxs_ap=diall[:, s0 * 8:(s0 + t) * 8], num_idxs=t * 128,
                    num_idxs_reg=t * 128, elem_size=ROWB,
                    single_packet=(t * 128 <= 1024), queue_num=ci % 2)

                # per-edge s_src via transposed one-hot; u = s_src + s_dst
                psU = psG.tile([128, MAXT * K], f32, tag="psU")
                for m in range(t):
                    nc.tensor.matmul(psU[:, m * K:(m + 1) * K],
                                     atb[:, m * 128:(m + 1) * 128], ssb[b][:],
                                     start=True, stop=True)
                del atb
                uv = sbG.tile([128, MAXT * K], f32, tag="uv")
                nc.vector.tensor_tensor(
                    out=uv[:, 0:t * K].rearrange("p (m k) -> p m k", k=K),
                    in0=psU[:, 0:t * K].rearrange("p (m k) -> p m k", k=K),
                    in1=gv[:, :, 512:528].bitcast(f32),
                    op=mybir.AluOpType.add)
                tv = sbG.tile([128, MAXT * K], f32, tag="tv")
                nc.vector.tensor_scalar_mul(tv[:, 0:t * K], uv[:, 0:t * K], ALPHA)
                lv = sbG.tile([128, MAXT * K], f32, tag="lv")
                nc.vector.tensor_max(lv[:, 0:t * K], uv[:, 0:t * K], tv[:, 0:t * K])
                wb = sbG.tile([128, MAXT * K], bf16, tag="wb")
                nc.scalar.activation(wb[:, 0:t * K], lv[:, 0:t * K], Exp,
                                     bias=shiftc[:])
                wbv = wb[:, 0:t * K].rearrange("p (m k) -> p m k", k=K)

                # gs per m: [w*h01 (256) | w0,w1,w0,w1 | w*h23 (256) | w2,w3,w2,w3]
                gs = sbG.tile([128, MAXT * 520], bf16, tag="gs")
                gsv = gs[:, 0:t * 520].rearrange("p (m r) -> p m r", r=520)
                nc.vector.tensor_tensor(
                    out=gsv[:, :, 0:256].rearrange("p m (k o) -> p m k o", o=128),
                    in0=gv[:, :, 0:256].rearrange("p m (k o) -> p m k o", o=128),
                    in1=wbv[:, :, 0:2, None].to_broadcast([128, t, 2, 128]),
                    op=mybir.AluOpType.mult)
                nc.vector.tensor_tensor(
                    out=gsv[:, :, 260:516].rearrange("p m (k o) -> p m k o", o=128),
                    in0=gv[:, :, 256:512].rearrange("p m (k o) -> p m k o", o=128),
                    in1=wbv[:, :, 2:4, None].to_broadcast([128, t, 2, 128]),
                    op=mybir.AluOpType.mult)
                nc.vector.tensor_copy(
                    out=gsv[:, :, 256:260].rearrange("p m (u v) -> p m u v", v=2),
                    in_=wbv[:, :, None, 0:2].to_broadcast([128, t, 2, 2]))
                nc.vector.tensor_copy(
                    out=gsv[:, :, 516:520].rearrange("p m (u v) -> p m u v", v=2),
                    in_=wbv[:, :, None, 2:4].to_broadcast([128, t, 2, 2]))

                if cl["first"]:
                    curA = psHI.tile([128, 260], f32, tag="hiA")
                    curB = psHI.tile([128, 260], f32, tag="hiB")
                for m in range(t):
                    st = cl["first"] and m == 0
                    sp = cl["last"] and m == t - 1
                    lhsT = asb[:, m * 128:(m + 1) * 128]
                    nc.tensor.matmul(curA[:], lhsT, gs[:, m * 520:m * 520 + 260],
                                     start=st, stop=sp)
                    nc.tensor.matmul(curB[:], lhsT,
                                     gs[:, m * 520 + 260:m * 520 + 520],
                                     start=st, stop=sp)
                if cl["last"]:
                    _finalize(nc, fin, b, curA, curB, xmall, emall, outall, Copy)
            nc.sync.dma_start(
                out=OUT[:, :].rearrange("(b p) f -> p b f", p=128),
                in_=outall[:].rearrange("p (b f) -> p b f", f=F))
    nc.finalize()
    return nc


def _finalize(nc, fin, b, hA, hB, xmall, emall, outall, Copy):
    xb = xmall[:, b * F:(b + 1) * F]
    eb = emall[:, b * K:(b + 1) * K]
    d4 = fin.tile([128, K], f32, tag="d4")
    nc.vector.tensor_scalar_add(d4[:, 0:2], hA[:, 256:258], EPS)
    nc.vector.tensor_scalar_add(d4[:, 2:4], hB[:, 256:258], EPS)
    r4 = fin.tile([128, K], f32, tag="r4")
    nc.vector.reciprocal(r4[:], d4[:])
    s4 = fin.tile([128, K], f32, tag="s4")
    nc.vector.tensor_mul(s4[:], r4[:], eb)
    t0 = fin.tile([128, F], f32, tag="t0")
    nc.vector.tensor_scalar_mul(t0[:], hA[:, 0:128], s4[:, 0:1])
    t1 = fin.tile([128, F], f32, tag="t1")
    nc.scalar.activation(t1[:], hA[:, 128:256], Copy, scale=s4[:, 1:2])
    t2 = fin.tile([128, F], f32, tag="t2")
    nc.vector.tensor_scalar_mul(t2[:], hB[:, 0:128], s4[:, 2:3])
    t3 = fin.tile([128, F], f32, tag="t3")
    nc.scalar.activation(t3[:], hB[:, 128:256], Copy, scale=s4[:, 3:4])
    q0 = fin.tile([128, F], f32, tag="q0")
    nc.vector.tensor_add(q0[:], t0[:], t1[:])
    q1 = fin.tile([128, F], f32, tag="q1")
    nc.vector.tensor_add(q1[:], t2[:], t3[:])
    q2 = fin.tile([128, F], f32, tag="q2")
    nc.vector.tensor_add(q2[:], q0[:], q1[:])
    nc.vector.tensor_add(outall[:, b * F:(b + 1) * F], q2[:], xb)


def kernel(x, e, weights, a, adj):
    meta, in_maps = _prep(np.asarray(x), np.asarray(e), np.asarray(weights),
                          np.asarray(a), np.asarray(adj))
    if meta["key"] not in _CACHE:
        _CACHE[meta["key"]] = _build(meta)
    nc = _CACHE[meta["key"]]
    res = run_bass_kernel_spmd(nc, in_maps, list(range(NCORES)))
    percore = np.stack([res.results[c]["out"] for c in range(NCORES)])
    return np.ascontiguousarray(percore[meta["core_of"], meta["loc"]])


# revision 5
# speedup vs baseline: 1.0052x; 1.0052x over previous
"""CaNet GAT-style layer on 8 Trainium2 NeuronCores — v2 (fp8 + dma_gather).

Algorithm (matches the jax reference):
  h[k]   = x @ W[k]                      per-head projection
  s_src  = x @ (W[k] a_src[k]),  s_dst = x @ (W[k] a_dst[k])
  w_e    = exp(leakyrelu(s_src[src] + s_dst[dst]) - 2)   (softmax shift
           invariance; the -2 keeps w*h inside fp8-e4m3 range)
  hi[k,i] = sum_{e: src[e]=i} w_e * h[k, dst[e]]
  den[k,i]= sum_{e: src[e]=i} w_e
  out[i] = sum_k e[i,k] * hi[k,i]/den[k,i] + x[i]

v2 vs v1: node table in fp8 (768-B rows: 512 B h + 16 B s_dst f32 + pad),
edge rows fetched with batched dma_gather (one call per (block, window),
int16 indices; two 32768-row windows cover the 50048-row table), one-hot
AND transposed one-hot shipped from host in fp8, per-block s_src
recomputed on the PE from xmT @ (W a_src).
"""

import heapq
import math
import os
import sys
from contextlib import ExitStack

import numpy as np

for _p in ("/opt/trn_rl_repo", "/root/.axon_site/_ro/trn_rl_repo"):
    if os.path.isdir(_p) and _p not in sys.path:
        sys.path.insert(0, _p)

import ml_dtypes

import concourse.bass as bass
import concourse.mybir as mybir
import concourse.tile as tile
from concourse import bacc
from concourse.bass_utils import run_bass_kernel_spmd
from concourse.masks import make_identity

N = 50000
F = 128
K = 4
NCORES = 8
BLK = 128
NBLK = 49                    # blocks per core (49*128 = 6272 >= 6250)
NPAD = NBLK * BLK            # 6272
NCHUNK = math.ceil(N / 128)  # 391
TROWS = NCHUNK * 128         # 50048
ROWB = 768                   # fp8 bytes/row: [0:512) h, [512:528) s_dst f32[4]
WIN = 32768                  # dma_gather int16 index window
W1B = TROWS - WIN            # 17280 = base of window 1
ALPHA = 0.01
EPS = 1e-8
SHIFT = 2.0

f32 = mybir.dt.float32
bf16 = mybir.dt.bfloat16
i16 = mybir.dt.int16
f8 = mybir.dt.float8e4
np_f8 = ml_dtypes.float8_e4m3

_CACHE = {}


def _prep(x, e, weights, a, adj):
    """Host-side sharding/indexing prep. Only touches adj for structure;
    all floating point math happens on device (x is only cast to bf16)."""
    src = np.concatenate([np.asarray(adj[0]), np.arange(N, dtype=np.int64)])
    dst = np.concatenate([np.asarray(adj[1]), np.arange(N, dtype=np.int64)])

    # degree-balanced assignment of nodes to (core, block) bins
    NBINS = NCORES * NBLK
    deg = np.bincount(src, minlength=N)
    order0 = np.argsort(-deg, kind="stable")
    heap = [(0, b) for b in range(NBINS)]
    heapq.heapify(heap)
    bcounts = np.zeros(NBINS, dtype=np.int64)
    assign = np.empty(N, dtype=np.int64)
    pos = np.empty(N, dtype=np.int64)
    for nd in order0:
        while True:
            s_, b_ = heapq.heappop(heap)
            if bcounts[b_] < BLK:
                break
        assign[nd] = b_
        pos[nd] = bcounts[b_]
        bcounts[b_] += 1
        if bcounts[b_] < BLK:
            heapq.heappush(heap, (s_ + int(deg[nd]), b_))

    core_of_n = assign // NBLK
    loc = (assign % NBLK) * BLK + pos

    ecore = core_of_n[src]
    eblk = assign[src] % NBLK
    esrcrel = pos[src]
    # window class: 0 forced, 1 forced, 2 = flexible (overlap region)
    wclass = np.where(dst < W1B, 0, np.where(dst >= WIN, 1, 2)).astype(np.int8)

    # group edges by (core, block); within: forced-w0, flex, forced-w1
    wkey = np.where(wclass == 1, 2, np.where(wclass == 2, 1, 0)).astype(np.int8)
    order = np.lexsort((dst, wkey, eblk, ecore))
    g_core = ecore[order]
    g_blk = eblk[order]
    g_src = esrcrel[order]
    g_dst = dst[order]
    g_wc = wclass[order]

    key = g_core * NBLK + g_blk
    counts = np.bincount(key, minlength=NBINS)
    starts = np.zeros(NBINS + 1, dtype=np.int64)
    np.cumsum(counts, out=starts[1:])

    # per (core, block): n0 range [lo, lo+flex]
    lo_cb = np.zeros(NBINS, dtype=np.int64)
    fl_cb = np.zeros(NBINS, dtype=np.int64)
    for cb in range(NBINS):
        s0, s1 = starts[cb], starts[cb + 1]
        wc = g_wc[s0:s1]
        lo_cb[cb] = int((wc == 0).sum())
        fl_cb[cb] = int((wc == 2).sum())

    # uniform (t0, t1) per block slot across cores
    t0 = np.zeros(NBLK, dtype=np.int64)
    t1 = np.zeros(NBLK, dtype=np.int64)
    for b in range(NBLK):
        ns = counts[b::NBLK]
        los = lo_cb[b::NBLK]
        his = los + fl_cb[b::NBLK]
        T = int(math.ceil(ns.max() / 128))
        found = False
        while not found:
            # prefer balanced splits (bounds the per-call SBUF tile)
            for d in range(T + 1):
                for a0 in ({(T + 1) // 2 - d, (T + 1) // 2 + d} if d else
                           {(T + 1) // 2}):
                    if not (0 <= a0 <= T):
                        continue
                    a1 = T - a0
                    lof = np.maximum(los, ns - 128 * a1)
                    hif = np.minimum(his, 128 * a0)
                    if (lof <= hif).all():
                        t0[b], t1[b] = a0, a1
                        found = True
                        break
                if found:
                    break
            if not found:
                T += 1

    S_TOT = int((t0 + t1).sum())
    calls = []
    s0_off = 0
    for b in range(NBLK):
        present = [w for w, t in ((0, t0[b]), (1, t1[b])) if t > 0]
        for w in present:
            t = int(t0[b] if w == 0 else t1[b])
            calls.append(dict(b=b, w=w, t=t, s0=s0_off,
                              first=(w == present[0]), last=(w == present[-1])))
            s0_off += t
    max_t = int(max(t0.max(), t1.max()))

    Ah = np.zeros((NCORES, 128, S_TOT * 128), dtype=np_f8)
    AhT = np.zeros((NCORES, 128, S_TOT * 128), dtype=np_f8)
    DI = np.zeros((NCORES, 128, S_TOT * 8), dtype=np.int16)

    sub_of = {(cl["b"], cl["w"]): cl["s0"] for cl in calls}
    for c in range(NCORES):
        for b in range(NBLK):
            cb = c * NBLK + b
            s0, s1 = starts[cb], starts[cb + 1]
            n = s1 - s0
            lof = max(lo_cb[cb], n - 128 * t1[b])
            hif = min(lo_cb[cb] + fl_cb[cb], 128 * t0[b])
            assert lof <= hif
            n0 = int(hif)  # maximize w0 fill
            for w, lo_e, hi_e in ((0, s0, s0 + n0), (1, s0 + n0, s1)):
                ne = hi_e - lo_e
                if ne == 0:
                    continue
                sb_ = sub_of[(b, w)]
                j = np.arange(ne)
                m = j // 128
                p = j % 128
                sr = g_src[lo_e:hi_e]
                dr = g_dst[lo_e:hi_e] - (0 if w == 0 else W1B)
                Ah[c, p, (sb_ + m) * 128 + sr] = 1.0
                AhT[c, sr, (sb_ + m) * 128 + p] = 1.0
                t = int(t0[b] if w == 0 else t1[b])
                il = np.zeros(t * 128, dtype=np.int16)
                il[m * 128 + p] = dr
                blk16 = il.reshape(t * 8, 16).T  # [16, t*8]
                DI[c, :, sb_ * 8:(sb_ + t) * 8] = np.tile(blk16, (8, 1))

    # combined per-call [asb | atb] regions: cols [s0*256, (s0+t)*256)
    AHB = np.zeros((NCORES, 128, S_TOT * 256), dtype=np_f8)
    for cl in calls:
        s0, t = cl["s0"], cl["t"]
        AHB[:, :, s0 * 256:s0 * 256 + t * 128] = \
            Ah[:, :, s0 * 128:(s0 + t) * 128]
        AHB[:, :, s0 * 256 + t * 128:(s0 + t) * 256] = \
            AhT[:, :, s0 * 128:(s0 + t) * 128]

    xtb = np.zeros((128, TROWS), dtype=ml_dtypes.bfloat16)
    xtb[:, :N] = x.T
    xm = np.zeros((NCORES, NPAD, F), dtype=np.float32)
    em = np.zeros((NCORES, NPAD, K), dtype=np.float32)
    xmt = np.zeros((NCORES, 128, NPAD), dtype=ml_dtypes.bfloat16)
    xm[core_of_n, loc] = x
    em[core_of_n, loc] = e
    for c in range(NCORES):
        xmt[c] = xm[c].T

    meta = dict(calls=calls, S_TOT=S_TOT, max_t=max_t,
                key=(tuple(int(v) for v in t0), tuple(int(v) for v in t1)),
                core_of=core_of_n, loc=loc)
    in_maps = []
    for c in range(NCORES):
        in_maps.append({
            "xtb": xtb,
            "w": np.ascontiguousarray(np.asarray(weights, dtype=np.float32)),
            "a": np.ascontiguousarray(np.asarray(a, dtype=np.float32)),
            "ahb": np.ascontiguousarray(AHB[c]),
            "di": np.ascontiguousarray(DI[c]),
            "xm": np.ascontiguousarray(xm[c]),
            "xmt": np.ascontiguousarray(xmt[c]),
            "em": np.ascontiguousarray(em[c]),
        })
    return meta, in_maps


def _build(meta, repeat=1):
    calls, S_TOT, MAXT = meta["calls"], meta["S_TOT"], meta["max_t"]
    nc = bacc.Bacc(num_swdge_queues=2)

    XTb = nc.declare_dram_parameter("xtb", [128, TROWS], bf16, isOutput=False)
    W = nc.declare_dram_parameter("w", [K, 128, 128], f32, isOutput=False)
    Aa = nc.declare_dram_parameter("a", [K, 256, 1], f32, isOutput=False)
    AHB = nc.declare_dram_parameter("ahb", [128, S_TOT * 256], f8, isOutput=False)
    DI = nc.declare_dram_parameter("di", [128, S_TOT * 8], i16, isOutput=False)
    XM = nc.declare_dram_parameter("xm", [NPAD, F], f32, isOutput=False)
    XMT = nc.declare_dram_parameter("xmt", [128, NPAD], bf16, isOutput=False)
    EM = nc.declare_dram_parameter("em", [NPAD, K], f32, isOutput=False)
    OUT = nc.declare_dram_parameter("out", [NPAD, F], f32, isOutput=True)

    Copy = mybir.ActivationFunctionType.Copy
    Exp = mybir.ActivationFunctionType.Exp
    Lrelu = mybir.ActivationFunctionType.Lrelu

    with ExitStack() as ctx:
        tc = ctx.enter_context(tile.TileContext(nc))
        if repeat > 1:
            # timing-only: repeat the whole body on-device (see bench3)
            ctx.enter_context(tc.For_i(0, repeat, name="rep"))
        const = ctx.enter_context(tc.tile_pool(name="const", bufs=1))
        sb = ctx.enter_context(tc.tile_pool(name="sb", bufs=4))
        sbG = ctx.enter_context(tc.tile_pool(name="sbG", bufs=4))
        fin = ctx.enter_context(tc.tile_pool(name="fin", bufs=3))
        dram = ctx.enter_context(tc.tile_pool(name="dram", bufs=1, space="DRAM"))

        Ttab = dram.tile([TROWS, ROWB], f8, tag="Ttab")

        ident = const.tile([128, 128], f32, tag="ident")
        make_identity(nc, ident[:])
        shiftc = const.tile([128, 1], f32, tag="shiftc")
        nc.gpsimd.memset(shiftc[:], -SHIFT)

        # ---- prologue: W01/W23 bf16, AVs/AVd = W_k @ a_{src,dst}[k], ssb ----
        W01 = const.tile([128, 256], bf16, tag="W01")
        W23AV = const.tile([128, 260], bf16, tag="W23AV")
        AVs = const.tile([128, K], bf16, tag="AVs")
        ssb = []
        with tc.tile_pool(name="psP", bufs=1, space="PSUM") as psP:
            for k in range(K):
                wk = sb.tile([128, 128], f32, tag="wk")
                nc.sync.dma_start(out=wk[:], in_=W[k])
                tgt = W01 if k < 2 else W23AV
                j = (k % 2) * 128
                nc.vector.tensor_copy(out=tgt[:, j:j + 128], in_=wk[:])
                ak = sb.tile([128, 2], f32, tag="ak")
                nc.sync.dma_start(out=ak[:, 0:1], in_=Aa[k, 0:128, :])
                nc.sync.dma_start(out=ak[:, 1:2], in_=Aa[k, 128:256, :])
                pT = psP.tile([128, 128], f32, tag="pT")
                nc.tensor.transpose(pT[:], wk[:], ident[:])
                wkT = sb.tile([128, 128], f32, tag="wkT")
                nc.scalar.copy(out=wkT[:], in_=pT[:])
                pva = psP.tile([128, 2], f32, tag="pva")
                nc.tensor.matmul(pva[:], wkT[:], ak[:], start=True, stop=True)
                nc.vector.tensor_copy(out=AVs[:, k:k + 1], in_=pva[:, 0:1])
                nc.vector.tensor_copy(out=W23AV[:, 256 + k:257 + k],
                                      in_=pva[:, 1:2])
            xmt = const.tile([128, NPAD], bf16, tag="xmt")
            nc.sync.dma_start(out=xmt[:], in_=XMT[:, :])
            diall = const.tile([128, S_TOT * 8], i16, tag="diall")
            nc.sync.dma_start(out=diall[:], in_=DI[:, :])
            xmall = const.tile([128, NBLK * F], f32, tag="xmall")
            nc.sync.dma_start(
                out=xmall[:].rearrange("p (b f) -> p b f", f=F),
                in_=XM[:, :].rearrange("(b p) f -> p b f", p=128))
            emall = const.tile([128, NBLK * K], f32, tag="emall")
            nc.sync.dma_start(
                out=emall[:].rearrange("p (b f) -> p b f", f=K),
                in_=EM[:, :].rearrange("(b p) f -> p b f", p=128))
            outall = const.tile([128, NBLK * F], f32, tag="outall")
            for b in range(NBLK):
                psS = psP.tile([128, K], f32, tag="psS")
                nc.tensor.matmul(psS[:], xmt[:, b * 128:(b + 1) * 128], AVs[:],
                                 start=True, stop=True)
                sb_b = const.tile([128, K], bf16, tag=f"ssb{b}")
                nc.scalar.copy(out=sb_b[:], in_=psS[:])
                ssb.append(sb_b)

        # ---- phase A: build node table Ttab (fp8 rows) ----
        with tc.tile_pool(name="psA", bufs=4, space="PSUM") as psA:
            for c4 in range(0, NCHUNK, 4):
                w4 = min(4, NCHUNK - c4)
                xc = sb.tile([128, 512], bf16, tag="xc")
                nc.scalar.dma_start(out=xc[:, 0:w4 * 128],
                                    in_=XTb[:, c4 * 128:(c4 + w4) * 128])
                tsb = sb.tile([128, 4 * ROWB], f8, tag="tsb")
                for cc in range(w4):
                    lhs = xc[:, cc * 128:(cc + 1) * 128]
                    o = cc * ROWB
                    pA = psA.tile([128, 256], f32, tag="pA")
                    pB = psA.tile([128, 260], f32, tag="pB")
                    nc.tensor.matmul(pA[:], lhs, W01[:], start=True, stop=True)
                    nc.tensor.matmul(pB[:], lhs, W23AV[:], start=True, stop=True)
                    nc.scalar.copy(out=tsb[:, o:o + 256], in_=pA[:])
                    nc.vector.tensor_copy(out=tsb[:, o + 256:o + 512],
                                          in_=pB[:, 0:256])
                    # s_dst f32[4] + pad filled with copies of it (one op)
                    nc.vector.tensor_copy(
                        out=tsb[:, o + 512:o + ROWB].bitcast(f32)
                            .rearrange("p (g f) -> p g f", f=4),
                        in_=pB[:, None, 256:260].to_broadcast([128, 16, 4]))
                nc.sync.dma_start(
                    out=Ttab[c4 * 128:(c4 + w4) * 128, :]
                        .rearrange("(q p) r -> p q r", p=128),
                    in_=tsb[:, 0:w4 * ROWB]
                        .rearrange("p (q r) -> p q r", r=ROWB))

        # ---- phase B: gather + attention + segment reduction ----
        with tc.tile_pool(name="psHI", bufs=2, space="PSUM") as psHI, \
                tc.tile_pool(name="psG", bufs=3, space="PSUM") as psG:
            curA = curB = None
            for ci, cl in enumerate(calls):
                b, w, t, s0 = cl["b"], cl["w"], cl["t"], cl["s0"]
                base = 0 if w == 0 else W1B
                graw = sbG.tile([128, MAXT * ROWB], f8, tag="graw")
                gv = graw[:, 0:t * ROWB].rearrange("p (m r) -> p m r", r=ROWB)
                nc.gpsimd.dma_gather(
                    out_ap=gv, in_ap=Ttab[base:base + WIN, :],
                    idxs_ap=diall[:, s0 * 8:(s0 + t) * 8], num_idxs=t * 128,
                    num_idxs_reg=t * 128, elem_size=ROWB,
                    single_packet=(t * 128 <= 1024), queue_num=ci % 2)
                ab = sbG.tile([128, MAXT * 256], f8, tag="ab")
                nc.sync.dma_start(out=ab[:, 0:t * 256],
                                  in_=AHB[:, s0 * 256:(s0 + t) * 256])
                asb = ab[:, 0:t * 128]
                atb = ab[:, t * 128:t * 256]

                # per-edge s_src via transposed one-hot; u = s_src + s_dst
                psU = psG.tile([128, MAXT * K], f32, tag="psU")
                for m in range(t):
                    nc.tensor.matmul(psU[:, m * K:(m + 1) * K],
                                     atb[:, m * 128:(m + 1) * 128], ssb[b][:],
                                     start=True, stop=True)
                del atb
                uv = sbG.tile([128, MAXT * K], f32, tag="uv")
                nc.vector.tensor_tensor(
                    out=uv[:, 0:t * K].rearrange("p (m k) -> p m k", k=K),
                    in0=psU[:, 0:t * K].rearrange("p (m k) -> p m k", k=K),
                    in1=gv[:, :, 512:528].bitcast(f32),
                    op=mybir.AluOpType.add)
                tv = sbG.tile([128, MAXT * K], f32, tag="tv")
                nc.vector.tensor_scalar_mul(tv[:, 0:t * K], uv[:, 0:t * K], ALPHA)
                lv = sbG.tile([128, MAXT * K], f32, tag="lv")
                nc.vector.tensor_max(lv[:, 0:t * K], uv[:, 0:t * K], tv[:, 0:t * K])
                wb = sbG.tile([128, MAXT * K], bf16, tag="wb")
                nc.scalar.activation(wb[:, 0:t * K], lv[:, 0:t * K], Exp,
                                     bias=shiftc[:])
                wbv = wb[:, 0:t * K].rearrange("p (m k) -> p m k", k=K)

                # gs per m: [w*h01 (256) | w0,w1,w0,w1 | w*h23 (256) | w2,w3,w2,w3]
                gs = sbG.tile([128, MAXT * 520], bf16, tag="gs")
                gsv = gs[:, 0:t * 520].rearrange("p (m r) -> p m r", r=520)
                nc.vector.tensor_tensor(
                    out=gsv[:, :, 0:256].rearrange("p m (k o) -> p m k o", o=128),
                    in0=gv[:, :, 0:256].rearrange("p m (k o) -> p m k o", o=128),
                    in1=wbv[:, :, 0:2, None].to_broadcast([128, t, 2, 128]),
                    op=mybir.AluOpType.mult)
                nc.vector.tensor_tensor(
                    out=gsv[:, :, 260:516].rearrange("p m (k o) -> p m k o", o=128),
                    in0=gv[:, :, 256:512].rearrange("p m (k o) -> p m k o", o=128),
                    in1=wbv[:, :, 2:4, None].to_broadcast([128, t, 2, 128]),
                    op=mybir.AluOpType.mult)
                nc.vector.tensor_copy(
                    out=gsv[:, :, 256:260].rearrange("p m (u v) -> p m u v", v=2),
                    in_=wbv[:, :, None, 0:2].to_broadcast([128, t, 2, 2]))
                nc.vector.tensor_copy(
                    out=gsv[:, :, 516:520].rearrange("p m (u v) -> p m u v", v=2),
                    in_=wbv[:, :, None, 2:4].to_broadcast([128, t, 2, 2]))

                if cl["first"]:
                    curA = psHI.tile([128, 260], f32, tag="hiA")
                    curB = psHI.tile([128, 260], f32, tag="hiB")
                for m in range(t):
                    st = cl["first"] and m == 0
                    sp = cl["last"] and m == t - 1
                    lhsT = asb[:, m * 128:(m + 1) * 128]
                    nc.tensor.matmul(curA[:], lhsT, gs[:, m * 520:m * 520 + 260],
                                     start=st, stop=sp)
                    nc.tensor.matmul(curB[:], lhsT,
                                     gs[:, m * 520 + 260:m * 520 + 520],
                                     start=st, stop=sp)
                if cl["last"]:
                    _finalize(nc, fin, b, curA, curB, xmall, emall, outall, Copy)
            nc.sync.dma_start(
                out=OUT[:, :].rearrange("(b p) f -> p b f", p=128),
                in_=outall[:].rearrange("p (b f) -> p b f", f=F))
    nc.finalize()
    return nc


def _finalize(nc, fin, b, hA, hB, xmall, emall, outall, Copy):
    xb = xmall[:, b * F:(b + 1) * F]
    eb = emall[:, b * K:(b + 1) * K]
    d4 = fin.tile([128, K], f32, tag="d4")
    nc.vector.tensor_scalar_add(d4[:, 0:2], hA[:, 256:258], EPS)
    nc.vector.tensor_scalar_add(d4[:, 2:4], hB[:, 256:258], EPS)
    r4 = fin.tile([128, K], f32, tag="r4")
    nc.vector.reciprocal(r4[:], d4[:])
    s4 = fin.tile([128, K], f32, tag="s4")
    nc.vector.tensor_mul(s4[:], r4[:], eb)
    t0 = fin.tile([128, F], f32, tag="t0")
    nc.vector.tensor_scalar_mul(t0[:], hA[:, 0:128], s4[:, 0:1])
    t1 = fin.tile([128, F], f32, tag="t1")
    nc.scalar.activation(t1[:], hA[:, 128:256], Copy, scale=s4[:, 1:2])
    t2 = fin.tile([128, F], f32, tag="t2")
    nc.vector.tensor_scalar_mul(t2[:], hB[:, 0:128], s4[:, 2:3])
    t3 = fin.tile([128, F], f32, tag="t3")
    nc.scalar.activation(t3[:], hB[:, 128:256], Copy, scale=s4[:, 3:4])
    q0 = fin.tile([128, F], f32, tag="q0")
    nc.vector.tensor_add(q0[:], t0[:], t1[:])
    q1 = fin.tile([128, F], f32, tag="q1")
    nc.vector.tensor_add(q1[:], t2[:], t3[:])
    q2 = fin.tile([128, F], f32, tag="q2")
    nc.vector.tensor_add(q2[:], q0[:], q1[:])
    nc.vector.tensor_add(outall[:, b * F:(b + 1) * F], q2[:], xb)


def kernel(x, e, weights, a, adj):
    meta, in_maps = _prep(np.asarray(x), np.asarray(e), np.asarray(weights),
                          np.asarray(a), np.asarray(adj))
    if meta["key"] not in _CACHE:
        _CACHE[meta["key"]] = _build(meta)
    nc = _CACHE[meta["key"]]
    res = run_bass_kernel_spmd(nc, in_maps, list(range(NCORES)))
    percore = np.stack([res.results[c]["out"] for c in range(NCORES)])
    return np.ascontiguousarray(percore[meta["core_of"], meta["loc"]])


# revision 8
# speedup vs baseline: 1.0171x; 1.0119x over previous
"""CaNet GAT-style layer on 8 Trainium2 NeuronCores — v2 (fp8 + dma_gather).

Algorithm (matches the jax reference):
  h[k]   = x @ W[k]                      per-head projection
  s_src  = x @ (W[k] a_src[k]),  s_dst = x @ (W[k] a_dst[k])
  w_e    = exp(leakyrelu(s_src[src] + s_dst[dst]) - 2)   (softmax shift
           invariance; the -2 keeps w*h inside fp8-e4m3 range)
  hi[k,i] = sum_{e: src[e]=i} w_e * h[k, dst[e]]
  den[k,i]= sum_{e: src[e]=i} w_e
  out[i] = sum_k e[i,k] * hi[k,i]/den[k,i] + x[i]

v2 vs v1: node table in fp8 (768-B rows: 512 B h + 16 B s_dst f32 + pad),
edge rows fetched with batched dma_gather (one call per (block, window),
int16 indices; two 32768-row windows cover the 50048-row table), one-hot
AND transposed one-hot shipped from host in fp8, per-block s_src
recomputed on the PE from xmT @ (W a_src).
"""

import heapq
import math
import os
import sys
from contextlib import ExitStack

import numpy as np

for _p in ("/opt/trn_rl_repo", "/root/.axon_site/_ro/trn_rl_repo"):
    if os.path.isdir(_p) and _p not in sys.path:
        sys.path.insert(0, _p)

import ml_dtypes

import concourse.bass as bass
import concourse.mybir as mybir
import concourse.tile as tile
from concourse import bacc
from concourse.bass_utils import run_bass_kernel_spmd
from concourse.masks import make_identity

N = 50000
F = 128
K = 4
NCORES = 8
BLK = 128
NBLK = 49                    # blocks per core (49*128 = 6272 >= 6250)
NPAD = NBLK * BLK            # 6272
NCHUNK = math.ceil(N / 128)  # 391
TROWS = NCHUNK * 128         # 50048
ROWB = 768                   # fp8 bytes/row: [0:512) h, [512:528) s_dst f32[4]
WIN = 32768                  # dma_gather int16 index window
W1B = TROWS - WIN            # 17280 = base of window 1
ALPHA = 0.01
EPS = 1e-8
SHIFT = 2.0

f32 = mybir.dt.float32
bf16 = mybir.dt.bfloat16
i16 = mybir.dt.int16
f8 = mybir.dt.float8e4
np_f8 = ml_dtypes.float8_e4m3

_CACHE = {}


def _prep(x, e, weights, a, adj):
    """Host-side sharding/indexing prep. Only touches adj for structure;
    all floating point math happens on device (x is only cast to bf16)."""
    src = np.concatenate([np.asarray(adj[0]), np.arange(N, dtype=np.int64)])
    dst = np.concatenate([np.asarray(adj[1]), np.arange(N, dtype=np.int64)])

    # degree-balanced assignment of nodes to (core, block) bins
    NBINS = NCORES * NBLK
    deg = np.bincount(src, minlength=N)
    order0 = np.argsort(-deg, kind="stable")
    heap = [(0, b) for b in range(NBINS)]
    heapq.heapify(heap)
    bcounts = np.zeros(NBINS, dtype=np.int64)
    assign = np.empty(N, dtype=np.int64)
    pos = np.empty(N, dtype=np.int64)
    for nd in order0:
        while True:
            s_, b_ = heapq.heappop(heap)
            if bcounts[b_] < BLK:
                break
        assign[nd] = b_
        pos[nd] = bcounts[b_]
        bcounts[b_] += 1
        if bcounts[b_] < BLK:
            heapq.heappush(heap, (s_ + int(deg[nd]), b_))

    core_of_n = assign // NBLK
    loc = (assign % NBLK) * BLK + pos

    ecore = core_of_n[src]
    eblk = assign[src] % NBLK
    esrcrel = pos[src]
    # window class: 0 forced, 1 forced, 2 = flexible (overlap region)
    wclass = np.where(dst < W1B, 0, np.where(dst >= WIN, 1, 2)).astype(np.int8)

    # group edges by (core, block); within: forced-w0, flex, forced-w1
    wkey = np.where(wclass == 1, 2, np.where(wclass == 2, 1, 0)).astype(np.int8)
    order = np.lexsort((dst, wkey, eblk, ecore))
    g_core = ecore[order]
    g_blk = eblk[order]
    g_src = esrcrel[order]
    g_dst = dst[order]
    g_wc = wclass[order]

    key = g_core * NBLK + g_blk
    counts = np.bincount(key, minlength=NBINS)
    starts = np.zeros(NBINS + 1, dtype=np.int64)
    np.cumsum(counts, out=starts[1:])

    # per (core, block): n0 range [lo, lo+flex]
    lo_cb = np.zeros(NBINS, dtype=np.int64)
    fl_cb = np.zeros(NBINS, dtype=np.int64)
    for cb in range(NBINS):
        s0, s1 = starts[cb], starts[cb + 1]
        wc = g_wc[s0:s1]
        lo_cb[cb] = int((wc == 0).sum())
        fl_cb[cb] = int((wc == 2).sum())

    # uniform (t0, t1) per block slot across cores
    t0 = np.zeros(NBLK, dtype=np.int64)
    t1 = np.zeros(NBLK, dtype=np.int64)
    for b in range(NBLK):
        ns = counts[b::NBLK]
        los = lo_cb[b::NBLK]
        his = los + fl_cb[b::NBLK]
        T = int(math.ceil(ns.max() / 128))
        found = False
        while not found:
            # prefer balanced splits (bounds the per-call SBUF tile)
            for d in range(T + 1):
                for a0 in ({(T + 1) // 2 - d, (T + 1) // 2 + d} if d else
                           {(T + 1) // 2}):
                    if not (0 <= a0 <= T):
                        continue
                    a1 = T - a0
                    lof = np.maximum(los, ns - 128 * a1)
                    hif = np.minimum(his, 128 * a0)
                    if (lof <= hif).all():
                        t0[b], t1[b] = a0, a1
                        found = True
                        break
                if found:
                    break
            if not found:
                T += 1

    S_TOT = int((t0 + t1).sum())
    calls = []
    s0_off = 0
    for b in range(NBLK):
        present = [w for w, t in ((0, t0[b]), (1, t1[b])) if t > 0]
        for w in present:
            t = int(t0[b] if w == 0 else t1[b])
            calls.append(dict(b=b, w=w, t=t, s0=s0_off,
                              first=(w == present[0]), last=(w == present[-1])))
            s0_off += t
    max_t = int(max(t0.max(), t1.max()))

    Ah = np.zeros((NCORES, 128, S_TOT * 128), dtype=np_f8)
    AhT = np.zeros((NCORES, 128, S_TOT * 128), dtype=np_f8)
    DI = np.zeros((NCORES, 128, S_TOT * 8), dtype=np.int16)

    sub_of = {(cl["b"], cl["w"]): cl["s0"] for cl in calls}
    for c in range(NCORES):
        for b in range(NBLK):
            cb = c * NBLK + b
            s0, s1 = starts[cb], starts[cb + 1]
            n = s1 - s0
            lof = max(lo_cb[cb], n - 128 * t1[b])
            hif = min(lo_cb[cb] + fl_cb[cb], 128 * t0[b])
            assert lof <= hif
            n0 = int(hif)  # maximize w0 fill
            for w, lo_e, hi_e in ((0, s0, s0 + n0), (1, s0 + n0, s1)):
                ne = hi_e - lo_e
                if ne == 0:
                    continue
                sb_ = sub_of[(b, w)]
                j = np.arange(ne)
                m = j // 128
                p = j % 128
                sr = g_src[lo_e:hi_e]
                dr = g_dst[lo_e:hi_e] - (0 if w == 0 else W1B)
                Ah[c, p, (sb_ + m) * 128 + sr] = 1.0
                AhT[c, sr, (sb_ + m) * 128 + p] = 1.0
                t = int(t0[b] if w == 0 else t1[b])
                il = np.zeros(t * 128, dtype=np.int16)
                il[m * 128 + p] = dr
                blk16 = il.reshape(t * 8, 16).T  # [16, t*8]
                DI[c, :, sb_ * 8:(sb_ + t) * 8] = np.tile(blk16, (8, 1))

    # combined per-call [asb | atb] regions: cols [s0*256, (s0+t)*256)
    AHB = np.zeros((NCORES, 128, S_TOT * 256), dtype=np_f8)
    for cl in calls:
        s0, t = cl["s0"], cl["t"]
        AHB[:, :, s0 * 256:s0 * 256 + t * 128] = \
            Ah[:, :, s0 * 128:(s0 + t) * 128]
        AHB[:, :, s0 * 256 + t * 128:(s0 + t) * 256] = \
            AhT[:, :, s0 * 128:(s0 + t) * 128]

    xtb = np.zeros((128, TROWS), dtype=ml_dtypes.bfloat16)
    xtb[:, :N] = x.T
    xm = np.zeros((NCORES, NPAD, F), dtype=np.float32)
    em = np.zeros((NCORES, NPAD, K), dtype=np.float32)
    xmt = np.zeros((NCORES, 128, NPAD), dtype=ml_dtypes.bfloat16)
    xm[core_of_n, loc] = x
    em[core_of_n, loc] = e
    for c in range(NCORES):
        xmt[c] = xm[c].T

    meta = dict(calls=calls, S_TOT=S_TOT, max_t=max_t,
                key=(tuple(int(v) for v in t0), tuple(int(v) for v in t1)),
                core_of=core_of_n, loc=loc)
    in_maps = []
    for c in range(NCORES):
        in_maps.append({
            "xtb": xtb,
            "w": np.ascontiguousarray(np.asarray(weights, dtype=np.float32)),
            "a": np.ascontiguousarray(np.asarray(a, dtype=np.float32)),
            "ahb": np.ascontiguousarray(AHB[c]),
            "di": np.ascontiguousarray(DI[c]),
            "xm": np.ascontiguousarray(xm[c]),
            "xmt": np.ascontiguousarray(xmt[c]),
            "em": np.ascontiguousarray(em[c]),
        })
    return meta, in_maps


def _build(meta, repeat=1):
    calls, S_TOT, MAXT = meta["calls"], meta["S_TOT"], meta["max_t"]
    nc = bacc.Bacc(num_swdge_queues=2)

    XTb = nc.declare_dram_parameter("xtb", [128, TROWS], bf16, isOutput=False)
    W = nc.declare_dram_parameter("w", [K, 128, 128], f32, isOutput=False)
    Aa = nc.declare_dram_parameter("a", [K, 256, 1], f32, isOutput=False)
    AHB = nc.declare_dram_parameter("ahb", [128, S_TOT * 256], f8, isOutput=False)
    DI = nc.declare_dram_parameter("di", [128, S_TOT * 8], i16, isOutput=False)
    XM = nc.declare_dram_parameter("xm", [NPAD, F], f32, isOutput=False)
    XMT = nc.declare_dram_parameter("xmt", [128, NPAD], bf16, isOutput=False)
    EM = nc.declare_dram_parameter("em", [NPAD, K], f32, isOutput=False)
    OUT = nc.declare_dram_parameter("out", [NPAD, F], f32, isOutput=True)

    Copy = mybir.ActivationFunctionType.Copy
    Exp = mybir.ActivationFunctionType.Exp
    Lrelu = mybir.ActivationFunctionType.Lrelu

    with ExitStack() as ctx:
        tc = ctx.enter_context(tile.TileContext(nc))
        if repeat > 1:
            # timing-only: repeat the whole body on-device (see bench3)
            ctx.enter_context(tc.For_i(0, repeat, name="rep"))
        const = ctx.enter_context(tc.tile_pool(name="const", bufs=1))
        sb = ctx.enter_context(tc.tile_pool(name="sb", bufs=3))
        sbG = ctx.enter_context(tc.tile_pool(name="sbG", bufs=4))
        fin = ctx.enter_context(tc.tile_pool(name="fin", bufs=2))
        dram = ctx.enter_context(tc.tile_pool(name="dram", bufs=1, space="DRAM"))

        Ttab = dram.tile([TROWS, ROWB], f8, tag="Ttab")

        ident = const.tile([128, 128], f32, tag="ident")
        make_identity(nc, ident[:])
        shiftc = const.tile([128, 1], f32, tag="shiftc")
        nc.gpsimd.memset(shiftc[:], -SHIFT)

        # ---- prologue: W01/W23 bf16, AVs/AVd = W_k @ a_{src,dst}[k], ssb ----
        W01 = const.tile([128, 256], bf16, tag="W01")
        W23AV = const.tile([128, 260], bf16, tag="W23AV")
        AVs = const.tile([128, K], bf16, tag="AVs")
        ssb = []
        with tc.tile_pool(name="psP", bufs=1, space="PSUM") as psP:
            for k in range(K):
                wk = sb.tile([128, 128], f32, tag="wk")
                nc.sync.dma_start(out=wk[:], in_=W[k])
                tgt = W01 if k < 2 else W23AV
                j = (k % 2) * 128
                nc.vector.tensor_copy(out=tgt[:, j:j + 128], in_=wk[:])
                ak = sb.tile([128, 2], f32, tag="ak")
                nc.sync.dma_start(out=ak[:, 0:1], in_=Aa[k, 0:128, :])
                nc.sync.dma_start(out=ak[:, 1:2], in_=Aa[k, 128:256, :])
                pT = psP.tile([128, 128], f32, tag="pT")
                nc.tensor.transpose(pT[:], wk[:], ident[:])
                wkT = sb.tile([128, 128], f32, tag="wkT")
                nc.scalar.copy(out=wkT[:], in_=pT[:])
                pva = psP.tile([128, 2], f32, tag="pva")
                nc.tensor.matmul(pva[:], wkT[:], ak[:], start=True, stop=True)
                nc.vector.tensor_copy(out=AVs[:, k:k + 1], in_=pva[:, 0:1])
                nc.vector.tensor_copy(out=W23AV[:, 256 + k:257 + k],
                                      in_=pva[:, 1:2])
            xmt = const.tile([128, NPAD], bf16, tag="xmt")
            nc.sync.dma_start(out=xmt[:], in_=XMT[:, :])
            diall = const.tile([128, S_TOT * 8], i16, tag="diall")
            nc.sync.dma_start(out=diall[:], in_=DI[:, :])
            xmall = const.tile([128, NBLK * F], f32, tag="xmall")
            nc.sync.dma_start(
                out=xmall[:].rearrange("p (b f) -> p b f", f=F),
                in_=XM[:, :].rearrange("(b p) f -> p b f", p=128))
            emall = const.tile([128, NBLK * K], f32, tag="emall")
            nc.sync.dma_start(
                out=emall[:].rearrange("p (b f) -> p b f", f=K),
                in_=EM[:, :].rearrange("(b p) f -> p b f", p=128))
            outall = const.tile([128, NBLK * F], f32, tag="outall")
            for b in range(NBLK):
                psS = psP.tile([128, K], f32, tag="psS")
                nc.tensor.matmul(psS[:], xmt[:, b * 128:(b + 1) * 128], AVs[:],
                                 start=True, stop=True)
                sb_b = const.tile([128, K], bf16, tag=f"ssb{b}")
                nc.scalar.copy(out=sb_b[:], in_=psS[:])
                ssb.append(sb_b)

        # ---- phase A: build node table Ttab (fp8 rows) ----
        with tc.tile_pool(name="psA", bufs=4, space="PSUM") as psA:
            for c4 in range(0, NCHUNK, 8):
                w4 = min(8, NCHUNK - c4)
                xc = sb.tile([128, 1024], bf16, tag="xc")
                nc.scalar.dma_start(out=xc[:, 0:w4 * 128],
                                    in_=XTb[:, c4 * 128:(c4 + w4) * 128])
                tsb = sb.tile([128, 8 * ROWB], f8, tag="tsb")
                for cc in range(w4):
                    lhs = xc[:, cc * 128:(cc + 1) * 128]
                    o = cc * ROWB
                    pA = psA.tile([128, 256], f32, tag="pA")
                    pB = psA.tile([128, 260], f32, tag="pB")
                    nc.tensor.matmul(pA[:], lhs, W01[:], start=True, stop=True)
                    nc.tensor.matmul(pB[:], lhs, W23AV[:], start=True, stop=True)
                    nc.scalar.copy(out=tsb[:, o:o + 256], in_=pA[:])
                    nc.vector.tensor_copy(out=tsb[:, o + 256:o + 512],
                                          in_=pB[:, 0:256])
                    # s_dst f32[4] + pad filled with copies of it (one op)
                    nc.vector.tensor_copy(
                        out=tsb[:, o + 512:o + ROWB].bitcast(f32)
                            .rearrange("p (g f) -> p g f", f=4),
                        in_=pB[:, None, 256:260].to_broadcast([128, 16, 4]))
                nc.sync.dma_start(
                    out=Ttab[c4 * 128:(c4 + w4) * 128, :]
                        .rearrange("(q p) r -> p q r", p=128),
                    in_=tsb[:, 0:w4 * ROWB]
                        .rearrange("p (q r) -> p q r", r=ROWB))

        # ---- phase B: gather + attention + segment reduction ----
        with tc.tile_pool(name="psHI", bufs=2, space="PSUM") as psHI, \
                tc.tile_pool(name="psG", bufs=3, space="PSUM") as psG:
            curA = curB = None
            for ci, cl in enumerate(calls):
                b, w, t, s0 = cl["b"], cl["w"], cl["t"], cl["s0"]
                base = 0 if w == 0 else W1B
                graw = sbG.tile([128, MAXT * ROWB], f8, tag="graw")
                gv = graw[:, 0:t * ROWB].rearrange("p (m r) -> p m r", r=ROWB)
                nc.gpsimd.dma_gather(
                    out_ap=gv, in_ap=Ttab[base:base + WIN, :],
                    idxs_ap=diall[:, s0 * 8:(s0 + t) * 8], num_idxs=t * 128,
                    num_idxs_reg=t * 128, elem_size=ROWB,
                    single_packet=(t * 128 <= 1024), queue_num=ci % 2)
                ab = sbG.tile([128, MAXT * 256], f8, tag="ab")
                nc.sync.dma_start(out=ab[:, 0:t * 256],
                                  in_=AHB[:, s0 * 256:(s0 + t) * 256])
                asb = ab[:, 0:t * 128]
                atb = ab[:, t * 128:t * 256]

                # per-edge s_src via transposed one-hot; u = s_src + s_dst
                psU = psG.tile([128, MAXT * K], f32, tag="psU")
                for m in range(t):
                    nc.tensor.matmul(psU[:, m * K:(m + 1) * K],
                                     atb[:, m * 128:(m + 1) * 128], ssb[b][:],
                                     start=True, stop=True)
                del atb
                uv = sbG.tile([128, MAXT * K], f32, tag="uv")
                nc.vector.tensor_tensor(
                    out=uv[:, 0:t * K].rearrange("p (m k) -> p m k", k=K),
                    in0=psU[:, 0:t * K].rearrange("p (m k) -> p m k", k=K),
                    in1=gv[:, :, 512:528].bitcast(f32),
                    op=mybir.AluOpType.add)
                tv = sbG.tile([128, MAXT * K], f32, tag="tv")
                nc.vector.tensor_scalar_mul(tv[:, 0:t * K], uv[:, 0:t * K], ALPHA)
                lv = sbG.tile([128, MAXT * K], f32, tag="lv")
                nc.vector.tensor_max(lv[:, 0:t * K], uv[:, 0:t * K], tv[:, 0:t * K])
                wb = sbG.tile([128, MAXT * K], bf16, tag="wb")
                nc.scalar.activation(wb[:, 0:t * K], lv[:, 0:t * K], Exp,
                                     bias=shiftc[:])
                wbv = wb[:, 0:t * K].rearrange("p (m k) -> p m k", k=K)

                # gs per m: [w*h01 (256) | w0,w1,w0,w1 | w*h23 (256) | w2,w3,w2,w3]
                gs = sbG.tile([128, MAXT * 520], bf16, tag="gs")
                gsv = gs[:, 0:t * 520].rearrange("p (m r) -> p m r", r=520)
                nc.vector.tensor_tensor(
                    out=gsv[:, :, 0:256].rearrange("p m (k o) -> p m k o", o=128),
                    in0=gv[:, :, 0:256].rearrange("p m (k o) -> p m k o", o=128),
                    in1=wbv[:, :, 0:2, None].to_broadcast([128, t, 2, 128]),
                    op=mybir.AluOpType.mult)
                nc.vector.tensor_tensor(
                    out=gsv[:, :, 260:516].rearrange("p m (k o) -> p m k o", o=128),
                    in0=gv[:, :, 256:512].rearrange("p m (k o) -> p m k o", o=128),
                    in1=wbv[:, :, 2:4, None].to_broadcast([128, t, 2, 128]),
                    op=mybir.AluOpType.mult)
                nc.vector.tensor_copy(
                    out=gsv[:, :, 256:260].rearrange("p m (u v) -> p m u v", v=2),
                    in_=wbv[:, :, None, 0:2].to_broadcast([128, t, 2, 2]))
                nc.vector.tensor_copy(
                    out=gsv[:, :, 516:520].rearrange("p m (u v) -> p m u v", v=2),
                    in_=wbv[:, :, None, 2:4].to_broadcast([128, t, 2, 2]))

                if cl["first"]:
                    curA = psHI.tile([128, 260], f32, tag="hiA")
                    curB = psHI.tile([128, 260], f32, tag="hiB")
                for m in range(t):
                    st = cl["first"] and m == 0
                    sp = cl["last"] and m == t - 1
                    lhsT = asb[:, m * 128:(m + 1) * 128]
                    nc.tensor.matmul(curA[:], lhsT, gs[:, m * 520:m * 520 + 260],
                                     start=st, stop=sp)
                    nc.tensor.matmul(curB[:], lhsT,
                                     gs[:, m * 520 + 260:m * 520 + 520],
                                     start=st, stop=sp)
                if cl["last"]:
                    _finalize(nc, fin, b, curA, curB, xmall, emall, outall, Copy)
            nc.sync.dma_start(
                out=OUT[:, :].rearrange("(b p) f -> p b f", p=128),
                in_=outall[:].rearrange("p (b f) -> p b f", f=F))
    nc.finalize()
    return nc


def _finalize(nc, fin, b, hA, hB, xmall, emall, outall, Copy):
    xb = xmall[:, b * F:(b + 1) * F]
    eb = emall[:, b * K:(b + 1) * K]
    d4 = fin.tile([128, K], f32, tag="d4")
    nc.vector.tensor_scalar_add(d4[:, 0:2], hA[:, 256:258], EPS)
    nc.vector.tensor_scalar_add(d4[:, 2:4], hB[:, 256:258], EPS)
    r4 = fin.tile([128, K], f32, tag="r4")
    nc.vector.reciprocal(r4[:], d4[:])
    s4 = fin.tile([128, K], f32, tag="s4")
    nc.vector.tensor_mul(s4[:], r4[:], eb)
    t0 = fin.tile([128, F], f32, tag="t0")
    nc.vector.tensor_scalar_mul(t0[:], hA[:, 0:128], s4[:, 0:1])
    t1 = fin.tile([128, F], f32, tag="t1")
    nc.scalar.activation(t1[:], hA[:, 128:256], Copy, scale=s4[:, 1:2])
    t2 = fin.tile([128, F], f32, tag="t2")
    nc.vector.tensor_scalar_mul(t2[:], hB[:, 0:128], s4[:, 2:3])
    t3 = fin.tile([128, F], f32, tag="t3")
    nc.scalar.activation(t3[:], hB[:, 128:256], Copy, scale=s4[:, 3:4])
    q0 = fin.tile([128, F], f32, tag="q0")
    nc.vector.tensor_add(q0[:], t0[:], t1[:])
    q1 = fin.tile([128, F], f32, tag="q1")
    nc.vector.tensor_add(q1[:], t2[:], t3[:])
    q2 = fin.tile([128, F], f32, tag="q2")
    nc.vector.tensor_add(q2[:], q0[:], q1[:])
    nc.vector.tensor_add(outall[:, b * F:(b + 1) * F], q2[:], xb)


def kernel(x, e, weights, a, adj):
    meta, in_maps = _prep(np.asarray(x), np.asarray(e), np.asarray(weights),
                          np.asarray(a), np.asarray(adj))
    if meta["key"] not in _CACHE:
        _CACHE[meta["key"]] = _build(meta)
    nc = _CACHE[meta["key"]]
    res = run_bass_kernel_spmd(nc, in_maps, list(range(NCORES)))
    percore = np.stack([res.results[c]["out"] for c in range(NCORES)])
    return np.ascontiguousarray(percore[meta["core_of"], meta["loc"]])


# revision 9
# speedup vs baseline: 1.0697x; 1.0517x over previous
"""CaNet GAT-style layer on 8 Trainium2 NeuronCores — v2 (fp8 + dma_gather).

Algorithm (matches the jax reference):
  h[k]   = x @ W[k]                      per-head projection
  s_src  = x @ (W[k] a_src[k]),  s_dst = x @ (W[k] a_dst[k])
  w_e    = exp(leakyrelu(s_src[src] + s_dst[dst]) - 2)   (softmax shift
           invariance; the -2 keeps w*h inside fp8-e4m3 range)
  hi[k,i] = sum_{e: src[e]=i} w_e * h[k, dst[e]]
  den[k,i]= sum_{e: src[e]=i} w_e
  out[i] = sum_k e[i,k] * hi[k,i]/den[k,i] + x[i]

v2 vs v1: node table in fp8 (768-B rows: 512 B h + 16 B s_dst f32 + pad),
edge rows fetched with batched dma_gather (one call per (block, window),
int16 indices; two 32768-row windows cover the 50048-row table), one-hot
AND transposed one-hot shipped from host in fp8, per-block s_src
recomputed on the PE from xmT @ (W a_src).
"""

import heapq
import math
import os
import sys
from contextlib import ExitStack

import numpy as np

for _p in ("/opt/trn_rl_repo", "/root/.axon_site/_ro/trn_rl_repo"):
    if os.path.isdir(_p) and _p not in sys.path:
        sys.path.insert(0, _p)

import ml_dtypes

import concourse.bass as bass
import concourse.mybir as mybir
import concourse.tile as tile
from concourse import bacc
from concourse.bass_utils import run_bass_kernel_spmd
from concourse.masks import make_identity

N = 50000
F = 128
K = 4
NCORES = 8
BLK = 128
NBLK = 49                    # blocks per core (49*128 = 6272 >= 6250)
NPAD = NBLK * BLK            # 6272
NCHUNK = math.ceil(N / 128)  # 391
TROWS = NCHUNK * 128         # 50048
ROWB = 768                   # fp8 bytes/row: [0:512) h, [512:528) s_dst f32[4]
WIN = 32768                  # dma_gather int16 index window
W1B = TROWS - WIN            # 17280 = base of window 1
ALPHA = 0.01
EPS = 1e-8
SHIFT = 2.0

f32 = mybir.dt.float32
bf16 = mybir.dt.bfloat16
i16 = mybir.dt.int16
f8 = mybir.dt.float8e4
np_f8 = ml_dtypes.float8_e4m3

_CACHE = {}


def _prep(x, e, weights, a, adj):
    """Host-side sharding/indexing prep. Only touches adj for structure;
    all floating point math happens on device (x is only cast to bf16)."""
    src = np.concatenate([np.asarray(adj[0]), np.arange(N, dtype=np.int64)])
    dst = np.concatenate([np.asarray(adj[1]), np.arange(N, dtype=np.int64)])

    # degree-balanced assignment of nodes to (core, block) bins
    NBINS = NCORES * NBLK
    deg = np.bincount(src, minlength=N)
    order0 = np.argsort(-deg, kind="stable")
    heap = [(0, b) for b in range(NBINS)]
    heapq.heapify(heap)
    bcounts = np.zeros(NBINS, dtype=np.int64)
    assign = np.empty(N, dtype=np.int64)
    pos = np.empty(N, dtype=np.int64)
    for nd in order0:
        while True:
            s_, b_ = heapq.heappop(heap)
            if bcounts[b_] < BLK:
                break
        assign[nd] = b_
        pos[nd] = bcounts[b_]
        bcounts[b_] += 1
        if bcounts[b_] < BLK:
            heapq.heappush(heap, (s_ + int(deg[nd]), b_))

    core_of_n = assign // NBLK
    loc = (assign % NBLK) * BLK + pos

    ecore = core_of_n[src]
    eblk = assign[src] % NBLK
    esrcrel = pos[src]
    # window class: 0 forced, 1 forced, 2 = flexible (overlap region)
    wclass = np.where(dst < W1B, 0, np.where(dst >= WIN, 1, 2)).astype(np.int8)

    # group edges by (core, block); within: forced-w0, flex, forced-w1
    wkey = np.where(wclass == 1, 2, np.where(wclass == 2, 1, 0)).astype(np.int8)
    order = np.lexsort((dst, wkey, eblk, ecore))
    g_core = ecore[order]
    g_blk = eblk[order]
    g_src = esrcrel[order]
    g_dst = dst[order]
    g_wc = wclass[order]

    key = g_core * NBLK + g_blk
    counts = np.bincount(key, minlength=NBINS)
    starts = np.zeros(NBINS + 1, dtype=np.int64)
    np.cumsum(counts, out=starts[1:])

    # per (core, block): n0 range [lo, lo+flex]
    lo_cb = np.zeros(NBINS, dtype=np.int64)
    fl_cb = np.zeros(NBINS, dtype=np.int64)
    for cb in range(NBINS):
        s0, s1 = starts[cb], starts[cb + 1]
        wc = g_wc[s0:s1]
        lo_cb[cb] = int((wc == 0).sum())
        fl_cb[cb] = int((wc == 2).sum())

    # uniform (t0, t1) per block slot across cores
    t0 = np.zeros(NBLK, dtype=np.int64)
    t1 = np.zeros(NBLK, dtype=np.int64)
    for b in range(NBLK):
        ns = counts[b::NBLK]
        los = lo_cb[b::NBLK]
        his = los + fl_cb[b::NBLK]
        T = int(math.ceil(ns.max() / 128))
        found = False
        while not found:
            # prefer balanced splits (bounds the per-call SBUF tile)
            for d in range(T + 1):
                for a0 in ({(T + 1) // 2 - d, (T + 1) // 2 + d} if d else
                           {(T + 1) // 2}):
                    if not (0 <= a0 <= T):
                        continue
                    a1 = T - a0
                    lof = np.maximum(los, ns - 128 * a1)
                    hif = np.minimum(his, 128 * a0)
                    if (lof <= hif).all():
                        t0[b], t1[b] = a0, a1
                        found = True
                        break
                if found:
                    break
            if not found:
                T += 1

    S_TOT = int((t0 + t1).sum())
    calls = []
    s0_off = 0
    for b in range(NBLK):
        present = [w for w, t in ((0, t0[b]), (1, t1[b])) if t > 0]
        for w in present:
            t = int(t0[b] if w == 0 else t1[b])
            calls.append(dict(b=b, w=w, t=t, s0=s0_off,
                              first=(w == present[0]), last=(w == present[-1])))
            s0_off += t
    max_t = int(max(t0.max(), t1.max()))

    Ah = np.zeros((NCORES, 128, S_TOT * 128), dtype=np_f8)
    AhT = np.zeros((NCORES, 128, S_TOT * 128), dtype=np_f8)
    DI = np.zeros((NCORES, 128, S_TOT * 8), dtype=np.int16)

    sub_of = {(cl["b"], cl["w"]): cl["s0"] for cl in calls}
    for c in range(NCORES):
        for b in range(NBLK):
            cb = c * NBLK + b
            s0, s1 = starts[cb], starts[cb + 1]
            n = s1 - s0
            lof = max(lo_cb[cb], n - 128 * t1[b])
            hif = min(lo_cb[cb] + fl_cb[cb], 128 * t0[b])
            assert lof <= hif
            n0 = int(hif)  # maximize w0 fill
            for w, lo_e, hi_e in ((0, s0, s0 + n0), (1, s0 + n0, s1)):
                ne = hi_e - lo_e
                if ne == 0:
                    continue
                sb_ = sub_of[(b, w)]
                j = np.arange(ne)
                m = j // 128
                p = j % 128
                sr = g_src[lo_e:hi_e]
                dr = g_dst[lo_e:hi_e] - (0 if w == 0 else W1B)
                Ah[c, p, (sb_ + m) * 128 + sr] = 1.0
                AhT[c, sr, (sb_ + m) * 128 + p] = 1.0
                t = int(t0[b] if w == 0 else t1[b])
                il = np.zeros(t * 128, dtype=np.int16)
                il[m * 128 + p] = dr
                blk16 = il.reshape(t * 8, 16).T  # [16, t*8]
                DI[c, :, sb_ * 8:(sb_ + t) * 8] = np.tile(blk16, (8, 1))

    # combined per-call [asb | atb] regions: cols [s0*256, (s0+t)*256)
    AHB = np.zeros((NCORES, 128, S_TOT * 256), dtype=np_f8)
    for cl in calls:
        s0, t = cl["s0"], cl["t"]
        AHB[:, :, s0 * 256:s0 * 256 + t * 128] = \
            Ah[:, :, s0 * 128:(s0 + t) * 128]
        AHB[:, :, s0 * 256 + t * 128:(s0 + t) * 256] = \
            AhT[:, :, s0 * 128:(s0 + t) * 128]

    xtb = np.zeros((128, TROWS), dtype=ml_dtypes.bfloat16)
    xtb[:, :N] = x.T
    xm = np.zeros((NCORES, NPAD, F), dtype=np.float32)
    em = np.zeros((NCORES, NPAD, K), dtype=np.float32)
    xmt = np.zeros((NCORES, 128, NPAD), dtype=ml_dtypes.bfloat16)
    xm[core_of_n, loc] = x
    em[core_of_n, loc] = e
    for c in range(NCORES):
        xmt[c] = xm[c].T

    meta = dict(calls=calls, S_TOT=S_TOT, max_t=max_t,
                key=(tuple(int(v) for v in t0), tuple(int(v) for v in t1)),
                core_of=core_of_n, loc=loc)
    in_maps = []
    for c in range(NCORES):
        in_maps.append({
            "xtb": xtb,
            "w": np.ascontiguousarray(np.asarray(weights, dtype=np.float32)),
            "a": np.ascontiguousarray(np.asarray(a, dtype=np.float32)),
            "ahb": np.ascontiguousarray(AHB[c]),
            "di": np.ascontiguousarray(DI[c]),
            "xm": np.ascontiguousarray(xm[c]),
            "xmt": np.ascontiguousarray(xmt[c]),
            "em": np.ascontiguousarray(em[c]),
        })
    return meta, in_maps


def _build(meta, repeat=1):
    calls, S_TOT, MAXT = meta["calls"], meta["S_TOT"], meta["max_t"]
    nc = bacc.Bacc(num_swdge_queues=2)

    XTb = nc.declare_dram_parameter("xtb", [128, TROWS], bf16, isOutput=False)
    W = nc.declare_dram_parameter("w", [K, 128, 128], f32, isOutput=False)
    Aa = nc.declare_dram_parameter("a", [K, 256, 1], f32, isOutput=False)
    AHB = nc.declare_dram_parameter("ahb", [128, S_TOT * 256], f8, isOutput=False)
    DI = nc.declare_dram_parameter("di", [128, S_TOT * 8], i16, isOutput=False)
    XM = nc.declare_dram_parameter("xm", [NPAD, F], f32, isOutput=False)
    XMT = nc.declare_dram_parameter("xmt", [128, NPAD], bf16, isOutput=False)
    EM = nc.declare_dram_parameter("em", [NPAD, K], f32, isOutput=False)
    OUT = nc.declare_dram_parameter("out", [NPAD, F], f32, isOutput=True)

    Copy = mybir.ActivationFunctionType.Copy
    Exp = mybir.ActivationFunctionType.Exp
    Lrelu = mybir.ActivationFunctionType.Lrelu

    with ExitStack() as ctx:
        tc = ctx.enter_context(tile.TileContext(nc))
        if repeat > 1:
            # timing-only: repeat the whole body on-device (see bench3)
            ctx.enter_context(tc.For_i(0, repeat, name="rep"))
        const = ctx.enter_context(tc.tile_pool(name="const", bufs=1))
        sb = ctx.enter_context(tc.tile_pool(name="sb", bufs=3))
        sbG = ctx.enter_context(tc.tile_pool(name="sbG", bufs=4))
        fin = ctx.enter_context(tc.tile_pool(name="fin", bufs=2))
        dram = ctx.enter_context(tc.tile_pool(name="dram", bufs=1, space="DRAM"))

        Ttab = dram.tile([TROWS, ROWB], f8, tag="Ttab")

        ident = const.tile([128, 128], f32, tag="ident")
        make_identity(nc, ident[:])
        shiftc = const.tile([128, 1], f32, tag="shiftc")
        nc.gpsimd.memset(shiftc[:], -SHIFT)

        # ---- prologue: W01/W23 bf16, AVs/AVd = W_k @ a_{src,dst}[k], ssb ----
        W01 = const.tile([128, 256], bf16, tag="W01")
        W23AV = const.tile([128, 260], bf16, tag="W23AV")
        AVs = const.tile([128, K], bf16, tag="AVs")
        ssb = []
        with tc.tile_pool(name="psP", bufs=1, space="PSUM") as psP:
            for k in range(K):
                wk = sb.tile([128, 128], f32, tag="wk")
                nc.sync.dma_start(out=wk[:], in_=W[k])
                tgt = W01 if k < 2 else W23AV
                j = (k % 2) * 128
                nc.vector.tensor_copy(out=tgt[:, j:j + 128], in_=wk[:])
                ak = sb.tile([128, 2], f32, tag="ak")
                nc.sync.dma_start(out=ak[:, 0:1], in_=Aa[k, 0:128, :])
                nc.sync.dma_start(out=ak[:, 1:2], in_=Aa[k, 128:256, :])
                pT = psP.tile([128, 128], f32, tag="pT")
                nc.tensor.transpose(pT[:], wk[:], ident[:])
                wkT = sb.tile([128, 128], f32, tag="wkT")
                nc.scalar.copy(out=wkT[:], in_=pT[:])
                pva = psP.tile([128, 2], f32, tag="pva")
                nc.tensor.matmul(pva[:], wkT[:], ak[:], start=True, stop=True)
                nc.vector.tensor_copy(out=AVs[:, k:k + 1], in_=pva[:, 0:1])
                nc.vector.tensor_copy(out=W23AV[:, 256 + k:257 + k],
                                      in_=pva[:, 1:2])
            xmt = const.tile([128, NPAD], bf16, tag="xmt")
            nc.sync.dma_start(out=xmt[:], in_=XMT[:, :])
            diall = const.tile([128, S_TOT * 8], i16, tag="diall")
            nc.sync.dma_start(out=diall[:], in_=DI[:, :])
            xmall = const.tile([128, NBLK * F], f32, tag="xmall")
            nc.sync.dma_start(
                out=xmall[:].rearrange("p (b f) -> p b f", f=F),
                in_=XM[:, :].rearrange("(b p) f -> p b f", p=128))
            emall = const.tile([128, NBLK * K], f32, tag="emall")
            nc.sync.dma_start(
                out=emall[:].rearrange("p (b f) -> p b f", f=K),
                in_=EM[:, :].rearrange("(b p) f -> p b f", p=128))
            outall = const.tile([128, NBLK * F], f32, tag="outall")
            for b in range(NBLK):
                psS = psP.tile([128, K], f32, tag="psS")
                nc.tensor.matmul(psS[:], xmt[:, b * 128:(b + 1) * 128], AVs[:],
                                 start=True, stop=True)
                sb_b = const.tile([128, K], bf16, tag=f"ssb{b}")
                nc.scalar.copy(out=sb_b[:], in_=psS[:])
                ssb.append(sb_b)

        # ---- phase A: build node table Ttab (fp8 rows) ----
        with tc.tile_pool(name="psA", bufs=4, space="PSUM") as psA:
            for c4 in range(0, NCHUNK, 8):
                w4 = min(8, NCHUNK - c4)
                xc = sb.tile([128, 1024], bf16, tag="xc")
                nc.scalar.dma_start(out=xc[:, 0:w4 * 128],
                                    in_=XTb[:, c4 * 128:(c4 + w4) * 128])
                tsb = sb.tile([128, 8 * ROWB], f8, tag="tsb")
                for cc in range(w4):
                    lhs = xc[:, cc * 128:(cc + 1) * 128]
                    o = cc * ROWB
                    pA = psA.tile([128, 256], f32, tag="pA")
                    pB = psA.tile([128, 260], f32, tag="pB")
                    nc.tensor.matmul(pA[:], lhs, W01[:], start=True, stop=True)
                    nc.tensor.matmul(pB[:], lhs, W23AV[:], start=True, stop=True)
                    nc.scalar.copy(out=tsb[:, o:o + 256], in_=pA[:])
                    nc.vector.tensor_copy(out=tsb[:, o + 256:o + 512],
                                          in_=pB[:, 0:256])
                    # s_dst f32[4] + pad filled with copies of it (one op)
                    nc.vector.tensor_copy(
                        out=tsb[:, o + 512:o + ROWB].bitcast(f32)
                            .rearrange("p (g f) -> p g f", f=4),
                        in_=pB[:, None, 256:260].to_broadcast([128, 16, 4]))
                nc.sync.dma_start(
                    out=Ttab[c4 * 128:(c4 + w4) * 128, :]
                        .rearrange("(q p) r -> p q r", p=128),
                    in_=tsb[:, 0:w4 * ROWB]
                        .rearrange("p (q r) -> p q r", r=ROWB))

        # ---- phase B: gather + attention + segment reduction ----
        with tc.tile_pool(name="psHI", bufs=3, space="PSUM") as psHI, \
                tc.tile_pool(name="psG", bufs=2, space="PSUM") as psG:
            curA = curB = None
            for ci, cl in enumerate(calls):
                b, w, t, s0 = cl["b"], cl["w"], cl["t"], cl["s0"]
                base = 0 if w == 0 else W1B
                graw = sbG.tile([128, MAXT * ROWB], f8, tag="graw")
                gv = graw[:, 0:t * ROWB].rearrange("p (m r) -> p m r", r=ROWB)
                nc.gpsimd.dma_gather(
                    out_ap=gv, in_ap=Ttab[base:base + WIN, :],
                    idxs_ap=diall[:, s0 * 8:(s0 + t) * 8], num_idxs=t * 128,
                    num_idxs_reg=t * 128, elem_size=ROWB,
                    single_packet=(t * 128 <= 1024), queue_num=ci % 2)
                ab = sbG.tile([128, MAXT * 256], f8, tag="ab")
                nc.sync.dma_start(out=ab[:, 0:t * 256],
                                  in_=AHB[:, s0 * 256:(s0 + t) * 256])
                asb = ab[:, 0:t * 128]
                atb = ab[:, t * 128:t * 256]

                # per-edge s_src via transposed one-hot; u = s_src + s_dst
                psU = psG.tile([128, MAXT * K], f32, tag="psU")
                for m in range(t):
                    nc.tensor.matmul(psU[:, m * K:(m + 1) * K],
                                     atb[:, m * 128:(m + 1) * 128], ssb[b][:],
                                     start=True, stop=True)
                del atb
                uv = sbG.tile([128, MAXT * K], f32, tag="uv")
                nc.vector.tensor_tensor(
                    out=uv[:, 0:t * K].rearrange("p (m k) -> p m k", k=K),
                    in0=psU[:, 0:t * K].rearrange("p (m k) -> p m k", k=K),
                    in1=gv[:, :, 512:528].bitcast(f32),
                    op=mybir.AluOpType.add)
                tv = sbG.tile([128, MAXT * K], f32, tag="tv")
                nc.vector.tensor_scalar_mul(tv[:, 0:t * K], uv[:, 0:t * K], ALPHA)
                lv = sbG.tile([128, MAXT * K], f32, tag="lv")
                nc.vector.tensor_max(lv[:, 0:t * K], uv[:, 0:t * K], tv[:, 0:t * K])
                wb = sbG.tile([128, MAXT * K], bf16, tag="wb")
                nc.scalar.activation(wb[:, 0:t * K], lv[:, 0:t * K], Exp,
                                     bias=shiftc[:])
                wbv = wb[:, 0:t * K].rearrange("p (m k) -> p m k", k=K)

                # gs per m: [w*h01 (256) | w0,w1,w0,w1 | w*h23 (256) | w2,w3,w2,w3]
                gs = sbG.tile([128, MAXT * 520], bf16, tag="gs")
                gsv = gs[:, 0:t * 520].rearrange("p (m r) -> p m r", r=520)
                nc.vector.tensor_tensor(
                    out=gsv[:, :, 0:256].rearrange("p m (k o) -> p m k o", o=128),
                    in0=gv[:, :, 0:256].rearrange("p m (k o) -> p m k o", o=128),
                    in1=wbv[:, :, 0:2, None].to_broadcast([128, t, 2, 128]),
                    op=mybir.AluOpType.mult)
                nc.vector.tensor_tensor(
                    out=gsv[:, :, 260:516].rearrange("p m (k o) -> p m k o", o=128),
                    in0=gv[:, :, 256:512].rearrange("p m (k o) -> p m k o", o=128),
                    in1=wbv[:, :, 2:4, None].to_broadcast([128, t, 2, 128]),
                    op=mybir.AluOpType.mult)
                nc.gpsimd.tensor_copy(
                    out=gsv[:, :, 256:260].rearrange("p m (u v) -> p m u v", v=2),
                    in_=wbv[:, :, None, 0:2].to_broadcast([128, t, 2, 2]))
                nc.gpsimd.tensor_copy(
                    out=gsv[:, :, 516:520].rearrange("p m (u v) -> p m u v", v=2),
                    in_=wbv[:, :, None, 2:4].to_broadcast([128, t, 2, 2]))

                if cl["first"]:
                    curA = psHI.tile([128, 260], f32, tag="hiA")
                    curB = psHI.tile([128, 260], f32, tag="hiB")
                for m in range(t):
                    st = cl["first"] and m == 0
                    sp = cl["last"] and m == t - 1
                    lhsT = asb[:, m * 128:(m + 1) * 128]
                    nc.tensor.matmul(curA[:], lhsT, gs[:, m * 520:m * 520 + 260],
                                     start=st, stop=sp)
                    nc.tensor.matmul(curB[:], lhsT,
                                     gs[:, m * 520 + 260:m * 520 + 520],
                                     start=st, stop=sp)
                if cl["last"]:
                    _finalize(nc, fin, b, curA, curB, xmall, emall, outall, Copy)
            nc.sync.dma_start(
                out=OUT[:, :].rearrange("(b p) f -> p b f", p=128),
                in_=outall[:].rearrange("p (b f) -> p b f", f=F))
    nc.finalize()
    return nc


def _finalize(nc, fin, b, hA, hB, xmall, emall, outall, Copy):
    xb = xmall[:, b * F:(b + 1) * F]
    eb = emall[:, b * K:(b + 1) * K]
    r4 = fin.tile([128, K], f32, tag="r4")
    nc.vector.reciprocal(r4[:, 0:2], hA[:, 256:258])
    nc.vector.reciprocal(r4[:, 2:4], hB[:, 256:258])
    s4 = fin.tile([128, K], f32, tag="s4")
    nc.vector.tensor_mul(s4[:], r4[:], eb)
    t0 = fin.tile([128, F], f32, tag="t0")
    nc.vector.tensor_scalar_mul(t0[:], hA[:, 0:128], s4[:, 0:1])
    t1 = fin.tile([128, F], f32, tag="t1")
    nc.scalar.activation(t1[:], hA[:, 128:256], Copy, scale=s4[:, 1:2])
    t2 = fin.tile([128, F], f32, tag="t2")
    nc.vector.tensor_scalar_mul(t2[:], hB[:, 0:128], s4[:, 2:3])
    t3 = fin.tile([128, F], f32, tag="t3")
    nc.scalar.activation(t3[:], hB[:, 128:256], Copy, scale=s4[:, 3:4])
    q0 = fin.tile([128, F], f32, tag="q0")
    nc.vector.tensor_add(q0[:], t0[:], t1[:])
    q1 = fin.tile([128, F], f32, tag="q1")
    nc.vector.tensor_add(q1[:], t2[:], t3[:])
    q2 = fin.tile([128, F], f32, tag="q2")
    nc.vector.tensor_add(q2[:], q0[:], q1[:])
    nc.vector.tensor_add(outall[:, b * F:(b + 1) * F], q2[:], xb)


def kernel(x, e, weights, a, adj):
    meta, in_maps = _prep(np.asarray(x), np.asarray(e), np.asarray(weights),
                          np.asarray(a), np.asarray(adj))
    if meta["key"] not in _CACHE:
        _CACHE[meta["key"]] = _build(meta)
    nc = _CACHE[meta["key"]]
    res = run_bass_kernel_spmd(nc, in_maps, list(range(NCORES)))
    percore = np.stack([res.results[c]["out"] for c in range(NCORES)])
    return np.ascontiguousarray(percore[meta["core_of"], meta["loc"]])
